# revision 14
# baseline (speedup 1.0000x reference)
"""Trainium2 Bass kernel for nn_ADCLayer (GAT-style message passing).

Math (reference reduction):
  sj = X @ (Wv @ aw[:d]) + bv.aw[:d]          (per-column score, j axis)
  si = X @ (Wv @ aw[d:]) + bv.aw[d:] + ab     (per-row score, i axis)
  alpha = A * exp(leaky_relu(si[i] + sj[j]))  (unnormalized transition)
  T = alpha / rowsum(alpha)
  H = X@Wk0 + (T X)@Wk1 + (T^2 X)@Wk2 + sum_k bk[k]   (last ref hop is dead code)
  out = relu(H)

Device algebra (per core, partition=j layout everywhere, zero transposes of
anything big):
  alphaT[j, i] = A^T[j, i] * exp(lrelu(si[i] + sj[j]))      (RAW, bf16)
  r via ones-stationary matmul -> (1, I) free layout.
  hop A runs on RAW alphaT (not gated by normalization):
    G2 = recip_col x (alphaT.T @ P2) + bks   (recip_col from 8 tiny
         PE-transposes of r; normalization folded into the evacuation)
  -> pairwise AllGather of G2.
  alphaT normalized in place (free-axis broadcast reciprocal) during the
  collective window; hop B then accumulates straight over X@Wk0:
  out = relu( X@Wk0 + alphaT_n.T @ (P1 + G2) ).

Sharding: 8 cores = 4 batches x 2 row-halves. j axis PERMUTED per core
(own half first) so own j-tiles have uniform local indices; partner G2
half selected with per-core 0/1 mask inputs.
"""

import numpy as np

B, N, DIN, DOUT = 4, 2048, 512, 512
HALF = N // 2          # rows per core
NCORES = 8
JT = N // 128          # 16 j tiles
IT = HALF // 128       # 8 i tiles (also own j tiles)
DT = DIN // 128        # 4 d tiles

_CACHE = {}


def _build():
    import concourse.bacc as bacc
    import concourse.tile as tile
    import concourse.mybir as mybir
    from concourse.bass import ts
    from concourse.tile_rust import add_dep_helper

    f32 = mybir.dt.float32
    bf16 = mybir.dt.bfloat16
    AOP = mybir.AluOpType
    AF = mybir.ActivationFunctionType

    nc = bacc.Bacc("TRN2", target_bir_lowering=False, debug=False,
                   num_devices=NCORES)

    AT = nc.declare_dram_parameter("AT", [N, HALF], bf16, isOutput=False)
    XT = nc.declare_dram_parameter("XT", [DIN, N], bf16, isOutput=False)
    WK0 = nc.declare_dram_parameter("WK0", [DIN, DOUT], bf16, isOutput=False)
    WK1 = nc.declare_dram_parameter("WK1", [DIN, DOUT], bf16, isOutput=False)
    WK2 = nc.declare_dram_parameter("WK2", [DIN, DOUT], bf16, isOutput=False)
    SJT = nc.declare_dram_parameter("SJT", [128, JT], f32, isOutput=False)
    SIB = nc.declare_dram_parameter("SIB", [1, HALF], f32, isOutput=False)
    BKS = nc.declare_dram_parameter("BKS", [1, DOUT], f32, isOutput=False)
    MLO = nc.declare_dram_parameter("MLO", [1, 1], f32, isOutput=False)
    MHI = nc.declare_dram_parameter("MHI", [1, 1], f32, isOutput=False)
    OUT = nc.declare_dram_parameter("out", [HALF, DOUT], f32, isOutput=True)

    g2_in = nc.dram_tensor("g2_in", [HALF, DOUT], bf16)
    g2_all = nc.dram_tensor("g2_all", [N, DOUT], bf16)
    r_dram = nc.dram_tensor("r_dram", [1, HALF], f32)

    with tile.TileContext(nc) as tc:
        with tc.tile_pool(name="sb", bufs=1) as sb:
            # ---- DMA priority classes ---------------------------------
            clsA, clsB = [], []
            sjt = sb.tile([128, JT], f32, tag="sjt", bufs=1)
            clsA.append(nc.sync.dma_start(out=sjt[:], in_=SJT[:, :]))
            sib = sb.tile([128, HALF], f32, tag="sib", bufs=1)
            clsA.append(nc.sync.dma_start(
                out=sib[:], in_=SIB[:, :].to_broadcast((128, HALF))))
            at = []
            for jt in range(2):
                t = sb.tile([128, HALF], bf16, tag=f"at{jt}", bufs=1)
                clsA.append(nc.sync.dma_start(out=t[:], in_=AT[ts(jt, 128), :]))
                at.append(t)

            def chain(dma, deps):
                for p in deps:
                    add_dep_helper(dma.ins, p.ins, reason="dma priority")
                return dma

            xt = []
            for d in range(DT):
                t = sb.tile([128, N], bf16, tag=f"xt{d}", bufs=1)
                clsB.append(chain(nc.sync.dma_start(out=t[:], in_=XT[ts(d, 128), :]),
                                  clsA))
                xt.append(t)
            wk = {}
            for nm, src in (("wk2", WK2),):
                wk[nm] = []
                for d in range(DT):
                    t = sb.tile([128, DOUT], bf16, tag=f"{nm}_{d}", bufs=1)
                    clsB.append(chain(nc.sync.dma_start(out=t[:], in_=src[ts(d, 128), :]),
                                      clsA))
                    wk[nm].append(t)
            for nm, src in (("wk1", WK1), ("wk0", WK0)):
                wk[nm] = []
                for d in range(DT):
                    t = sb.tile([128, DOUT], bf16, tag=f"{nm}_{d}", bufs=1)
                    chain(nc.sync.dma_start(out=t[:], in_=src[ts(d, 128), :]), clsB)
                    wk[nm].append(t)
            for jt in range(2, JT):
                t = sb.tile([128, HALF], bf16, tag=f"at{jt}", bufs=1)
                chain(nc.sync.dma_start(out=t[:], in_=AT[ts(jt, 128), :]), clsB)
                at.append(t)
            bks = sb.tile([128, DOUT], f32, tag="bks", bufs=1)
            chain(nc.sync.dma_start(out=bks[:],
                                    in_=BKS[:, :].to_broadcast((128, DOUT))), clsB)
            mlo = sb.tile([128, 1], f32, tag="mlo", bufs=1)
            chain(nc.sync.dma_start(out=mlo[:],
                                    in_=MLO[:, :].to_broadcast((128, 1))), clsB)
            mhi = sb.tile([128, 1], f32, tag="mhi", bufs=1)
            chain(nc.sync.dma_start(out=mhi[:],
                                    in_=MHI[:, :].to_broadcast((128, 1))), clsB)
            ones = sb.tile([128, 1], bf16, tag="ones", bufs=1)
            nc.vector.memset(ones[:], 1.0)
            one1 = sb.tile([1, 1], f32, tag="one1", bufs=1)
            nc.vector.memset(one1[:], 1.0)

            g2sb = []
            with tc.tile_pool(name="psAll", bufs=1, space="PSUM") as psA:
                # ---- phase 1: elementwise alphaT (raw) + r + P2 --------
                r_ps = psA.tile([1, HALF], f32, tag="r", bufs=1)
                for jt in range(JT):
                    t = at[jt]
                    z = sb.tile([128, HALF], f32, tag="z", bufs=3)
                    nc.scalar.activation(z[:], sib[:], AF.Identity,
                                         bias=sjt[:, jt:jt + 1], scale=1.0)
                    nc.vector.scalar_tensor_tensor(z[:], z[:], 0.2, z[:],
                                                   op0=AOP.mult, op1=AOP.max)
                    e = sb.tile([128, HALF], bf16, tag="e", bufs=3)
                    nc.scalar.activation(e[:], z[:], AF.Exp)
                    nc.vector.tensor_mul(t[:], t[:], e[:])
                    for h in range(2):
                        nc.tensor.matmul(r_ps[:, ts(h, 512)], lhsT=ones[:],
                                         rhs=t[:, ts(h, 512)],
                                         start=(jt == 0), stop=(jt == JT - 1))

                p2 = []
                for jt in range(JT):
                    pp2 = psA.tile([128, DOUT], f32, tag="mm", bufs=2)
                    for d in range(DT):
                        nc.tensor.matmul(pp2[:], lhsT=xt[d][:, ts(jt, 128)],
                                         rhs=wk["wk2"][d][:],
                                         start=(d == 0), stop=(d == DT - 1))
                    t2 = sb.tile([128, DOUT], bf16, tag=f"p2_{jt}", bufs=1)
                    nc.vector.tensor_copy(t2[:], pp2[:])
                    p2.append(t2)

                # ---- hop A on RAW alphaT: two 4-bank passes ------------
                ua = [None] * IT
                for half in range(2):
                    for i in range(4):
                        it = half * 4 + i
                        ua[it] = psA.tile([128, DOUT], f32, tag=f"ua{i}",
                                          bufs=1, name=f"ua{half}_{i}")
                    for jt in range(JT):
                        for i in range(4):
                            it = half * 4 + i
                            nc.tensor.matmul(ua[it][:],
                                             lhsT=at[jt][:, ts(it, 128)],
                                             rhs=p2[jt][:],
                                             start=(jt == 0),
                                             stop=(jt == JT - 1))

                    # r -> per-partition recip_col via 8 tiny PE transposes
                    if half == 0:
                        r_sb = sb.tile([1, HALF], f32, tag="rsb", bufs=1)
                        nc.vector.tensor_copy(r_sb[:], r_ps[:])
                        nc.sync.dma_start(out=r_dram[:, :], in_=r_sb[:])
                        rt = psA.tile([128, IT], f32, tag="r", bufs=1,
                                      name="rt")
                        for c in range(IT):
                            nc.tensor.matmul(rt[:, c:c + 1],
                                             lhsT=r_sb[0:1, ts(c, 128)],
                                             rhs=one1[:],
                                             is_transpose=True,
                                             start=True, stop=True)
                        rr_col = sb.tile([128, IT], f32, tag="rrc", bufs=1)
                        nc.vector.reciprocal(rr_col[:], rt[:])
                        # free-axis reciprocal for the hop-B normalize
                        r128 = sb.tile([128, HALF], f32, tag="r128", bufs=1)
                        nc.sync.dma_start(
                            out=r128[:],
                            in_=r_dram[:, :].to_broadcast((128, HALF)))
                        rr128 = sb.tile([128, HALF], f32, tag="rr128", bufs=1)
                        nc.vector.reciprocal_approx_fast(rr128[:], r128[:])

                    # G2 = recip_col * UA + bks, straight to the gather
                    for i in range(4):
                        it = half * 4 + i
                        g2t = sb.tile([128, DOUT], bf16, tag=f"g2o{it}",
                                      bufs=1, name=f"g2o{it}")
                        nc.vector.scalar_tensor_tensor(
                            g2t[:], ua[it][:], rr_col[:, it:it + 1], bks[:],
                            op0=AOP.mult, op1=AOP.add)
                        g2sb.append(g2t)
                        nc.sync.dma_start(out=g2_in[ts(it, 128), :],
                                          in_=g2t[:])

            nc.gpsimd.collective_compute(
                "AllGather", AOP.bypass,
                ins=[g2_in.ap().opt()],
                outs=[g2_all.ap().opt()],
                replica_groups=[[0, 1], [2, 3], [4, 5], [6, 7]],
            )

            # ---- cover window: normalize alphaT, P1, S-own ------------
            for jt in range(JT):
                nc.vector.tensor_mul(at[jt][:], at[jt][:], rr128[:])
            p1 = []
            with tc.tile_pool(name="psM", bufs=1, space="PSUM") as psM:
                for jt in range(JT):
                    pp1 = psM.tile([128, DOUT], f32, tag="mm1", bufs=3)
                    for d in range(DT):
                        nc.tensor.matmul(pp1[:], lhsT=xt[d][:, ts(jt, 128)],
                                         rhs=wk["wk1"][d][:],
                                         start=(d == 0), stop=(d == DT - 1))
                    t1 = sb.tile([128, DOUT], bf16, tag=f"p1_{jt}", bufs=1)
                    if jt < IT:
                        nc.vector.scalar_tensor_tensor(t1[:], pp1[:], 1.0,
                                                       g2sb[jt][:],
                                                       op0=AOP.mult,
                                                       op1=AOP.add)
                    else:
                        nc.vector.tensor_copy(t1[:], pp1[:])
                    p1.append(t1)

            # ---- S-other: partner half via masked add -----------------
            for k in range(IT):
                glo = sb.tile([128, DOUT], bf16, tag=f"glo{k}", bufs=1,
                              name=f"glo{k}")
                nc.sync.dma_start(out=glo[:], in_=g2_all[ts(k, 128), :])
                ghi = sb.tile([128, DOUT], bf16, tag=f"ghi{k}", bufs=1,
                              name=f"ghi{k}")
                nc.sync.dma_start(out=ghi[:], in_=g2_all[ts(IT + k, 128), :])
                stmp = sb.tile([128, DOUT], bf16, tag="stmp", bufs=3)
                nc.vector.scalar_tensor_tensor(stmp[:], glo[:], mlo[:, 0:1],
                                               p1[IT + k][:],
                                               op0=AOP.mult, op1=AOP.add)
                nc.vector.scalar_tensor_tensor(p1[IT + k][:], ghi[:],
                                               mhi[:, 0:1], stmp[:],
                                               op0=AOP.mult, op1=AOP.add)

            # ---- phase 3: H = X@Wk0 + alphaT_n.T @ S ------------------
            with tc.tile_pool(name="psC", bufs=1, space="PSUM") as psC:
                hps = [psC.tile([128, DOUT], f32, tag=f"h{i}", bufs=1,
                               name=f"h{i}") for i in range(IT)]
                for it in range(IT):
                    for d in range(DT):
                        nc.tensor.matmul(hps[it][:],
                                         lhsT=xt[d][:, ts(it, 128)],
                                         rhs=wk["wk0"][d][:],
                                         start=(d == 0), stop=False)
                for jt in range(IT):
                    for it in range(IT):
                        nc.tensor.matmul(hps[it][:],
                                         lhsT=at[jt][:, ts(it, 128)],
                                         rhs=p1[jt][:],
                                         start=False, stop=False)
                for it_half in (range(0, IT // 2), range(IT // 2, IT)):
                    for it in it_half:
                        for jt in range(IT, JT):
                            nc.tensor.matmul(hps[it][:],
                                             lhsT=at[jt][:, ts(it, 128)],
                                             rhs=p1[jt][:],
                                             start=False,
                                             stop=(jt == JT - 1))
                        o = sb.tile([128, DOUT], f32, tag="osb", bufs=3)
                        nc.scalar.activation(o[:], hps[it][:], AF.Relu)
                        nc.sync.dma_start(out=OUT[ts(it, 128), :], in_=o[:])

    nc.compile()
    return nc


def _prep_inputs(X, A, Wv, bv, aw, ab, Wk, bk):
    import ml_dtypes

    bf16 = ml_dtypes.bfloat16
    X = np.asarray(X, np.float32)
    A = np.asarray(A, np.float32)
    Wv = np.asarray(Wv, np.float32)
    bv = np.asarray(bv, np.float32)
    aw = np.asarray(aw, np.float32)
    ab = np.asarray(ab, np.float32)
    Wk = np.asarray(Wk, np.float32)
    bk = np.asarray(bk, np.float32)

    w1 = Wv @ aw[:DOUT, 0]
    c1 = float(bv @ aw[:DOUT, 0])
    w2 = Wv @ aw[DOUT:, 0]
    c2 = float(bv @ aw[DOUT:, 0]) + float(ab[0])
    bks = bk.sum(axis=0).astype(np.float32)

    wk_b = [np.ascontiguousarray(Wk[k]).astype(bf16) for k in range(3)]
    in_maps = []
    for c in range(NCORES):
        b, hf = c // 2, c % 2
        own = slice(hf * HALF, (hf + 1) * HALF)
        oth = slice((1 - hf) * HALF, (2 - hf) * HALF)
        perm = np.r_[np.arange(own.start, own.stop),
                     np.arange(oth.start, oth.stop)]
        Xb = X[b]
        sj = (Xb @ w1 + c1).astype(np.float32)
        si = (Xb @ w2 + c2).astype(np.float32)
        in_maps.append({
            "AT": np.ascontiguousarray(A[b][own, :].T[perm, :]).astype(bf16),
            "XT": np.ascontiguousarray(Xb.T[:, perm]).astype(bf16),
            "WK0": wk_b[0], "WK1": wk_b[1], "WK2": wk_b[2],
            "SJT": np.ascontiguousarray(sj[perm].reshape(JT, 128).T,
                                        np.float32),
            "SIB": np.ascontiguousarray(si[own][None, :], np.float32),
            "BKS": np.ascontiguousarray(bks[None, :], np.float32),
            "MLO": np.full((1, 1), 1.0 if hf == 1 else 0.0, np.float32),
            "MHI": np.full((1, 1), 1.0 if hf == 0 else 0.0, np.float32),
        })
    return in_maps


LAST_RESULTS = None


def kernel(X, A, Wv, bv, aw, ab, Wk, bk):
    from concourse.bass_utils import run_bass_kernel_spmd

    if "nc" not in _CACHE:
        _CACHE["nc"] = _build()
    nc = _CACHE["nc"]

    in_maps = _prep_inputs(X, A, Wv, bv, aw, ab, Wk, bk)
    res = run_bass_kernel_spmd(nc, in_maps, core_ids=list(range(NCORES)))
    global LAST_RESULTS
    LAST_RESULTS = res

    out = np.empty((B, N, DOUT), np.float32)
    for c in range(NCORES):
        b, hf = c // 2, c % 2
        out[b, hf * HALF:(hf + 1) * HALF, :] = res.results[c]["out"]
    return out
